# revision 1
# baseline (speedup 1.0000x reference)
"""Trainium2 Bass kernel for the low-rank linear operator.

Math: the reference collapses algebraically. With y = linspace(-1,1,H),
x = linspace(-1,1,W), dx = 2/(W-1):

  Vy[b,i] = sum_{h,w} v[b,i,h,w] * y_h
  Vx[b,i] = sum_{h,w} v[b,i,h,w] * x_w
  inner[b,r] = dx * sum_i (Vy[b,i]*psi[r,i,0] + Vx[b,i]*psi[r,i,1])
  A[b,o] = sum_r inner[b,r]*phi[o,r,0];  Bc[b,o] = sum_r inner[b,r]*phi[o,r,1]
  u[b,o,h,w] = A[b,o]*y_h + Bc[b,o]*x_w

Sharding: data-parallel over batch, 2 batches per core, 8 cores, no
collectives.

Layout: two h-rows per partition (p = h//2, hh = h%2 on the free axis) so
every DMA descriptor moves 2KB contiguous (the HW profile showed 1KB
descriptors made DMA packet-rate, not bytes, the bottleneck). Since y is
affine (y[2p+hh] = y[2p] + hh*dy), PE matmuls against a [y_even, 1]
stationary still recover the h-weighted sums, with an hh==1 correction
folded into the second reduction pass. ACT/DVE drain psum; a DRAM bounce
re-partitions per-channel rows to [128(2i+p), (hh w)]; full-width DVE
mult+reduce passes produce the (Vy-part, Vx) vectors; tiny PE matmuls give
inner -> (A,B) broadcast into per-partition scale/bias; DVE/ACT/Pool
generate u tiles as x_w*B + y_even*A (+ A*dy on the hh=1 half).
"""

import sys

try:
    import concourse.bass as bass  # noqa: F401
except ImportError:
    for _p in ("/opt/trn_rl_repo", "/root/.axon_site/_ro/trn_rl_repo"):
        if _p not in sys.path:
            sys.path.insert(0, _p)

import numpy as np

import concourse.bacc as bacc
import concourse.bass as bass
import concourse.mybir as mybir
import concourse.tile as tile
from concourse.bass_utils import run_bass_kernel_spmd

F32 = mybir.dt.float32
MULT = mybir.AluOpType.mult
ADD = mybir.AluOpType.add

B, CI, CO, R, H, W = 16, 64, 64, 64, 256, 256
N_CORES = 8
BPC = B // N_CORES  # batches per core
HP = H // 2         # h-pairs per partition dim

# generation-engine rotation
_GEN_ENGINES = ("dve", "act", "dve", "act", "pool", "dve", "act", "pool")


def build_nc():
    nc = bacc.Bacc("TRN2", target_bir_lowering=False, debug=False)

    v = nc.dram_tensor("v", [BPC, CI, H, W], F32, kind="ExternalInput")
    psi2y = nc.dram_tensor("psi2y", [2 * CI, R], F32, kind="ExternalInput")
    psi2x = nc.dram_tensor("psi2x", [2 * CI, R], F32, kind="ExternalInput")
    phicat = nc.dram_tensor("phicat", [R, 2 * CO], F32, kind="ExternalInput")
    wty = nc.dram_tensor("wty", [2 * CI, 2 * W], F32, kind="ExternalInput")
    wtx = nc.dram_tensor("wtx", [2 * CI, 2 * W], F32, kind="ExternalInput")
    y2e = nc.dram_tensor("y2e", [HP, 2], F32, kind="ExternalInput")
    xrep = nc.dram_tensor("xrep", [128, W], F32, kind="ExternalInput")
    ybc = nc.dram_tensor("ybc", [1, 384], F32, kind="ExternalInput")
    ident1 = nc.dram_tensor("ident1", [1, 1], F32, kind="ExternalInput")
    u = nc.dram_tensor("u", [BPC, CO, H, W], F32, kind="ExternalOutput")

    IBLK = 8          # channels per input DMA
    NBLK = CI // IBLK
    OBLK = 4          # output channels per output DMA
    NOBLK = CO // OBLK

    with tile.TileContext(nc) as tc:
        with (
            tc.tile_pool(name="consts", bufs=1) as consts,
            tc.tile_pool(name="inp", bufs=3) as in_pool,
            tc.tile_pool(name="outp", bufs=4) as out_pool,
            tc.tile_pool(name="scr", bufs=3) as scratch,
            tc.tile_pool(name="bc", bufs=6) as bc_pool,
            tc.tile_pool(name="psumP", bufs=5, space="PSUM") as psum_p,
            tc.tile_pool(name="psumT", bufs=1, space="PSUM") as psum_t,
            tc.tile_pool(name="psumBC", bufs=2, space="PSUM") as psum_bc,
            tc.tile_pool(name="dram", bufs=2, space="DRAM") as dram_pool,
        ):
            sb_psi2y = consts.tile([2 * CI, R], F32)
            nc.scalar.dma_start(sb_psi2y[:], psi2y[:])
            sb_psi2x = consts.tile([2 * CI, R], F32)
            nc.scalar.dma_start(sb_psi2x[:], psi2x[:])
            sb_phicat = consts.tile([R, 2 * CO], F32)
            nc.scalar.dma_start(sb_phicat[:], phicat[:])
            sb_wty = consts.tile([2 * CI, 2 * W], F32)
            nc.scalar.dma_start(sb_wty[:], wty[:])
            sb_wtx = consts.tile([2 * CI, 2 * W], F32)
            nc.scalar.dma_start(sb_wtx[:], wtx[:])
            sb_y2e = consts.tile([HP, 2], F32)
            nc.scalar.dma_start(sb_y2e[:], y2e[:])
            sb_xrep = consts.tile([128, W], F32)
            nc.scalar.dma_start(sb_xrep[:], xrep[:])
            sb_ybc = consts.tile([1, 384], F32)
            nc.scalar.dma_start(sb_ybc[:], ybc[:])
            sb_id1 = consts.tile([1, 1], F32)
            nc.scalar.dma_start(sb_id1[:], ident1[:])

            # reduction vectors: partition 2i   -> y-part (needs pair-sum)
            #                    partition 2i+1 -> correction / Vx
            gcaty = consts.tile([2 * CI, BPC], F32)
            gcatx = consts.tile([2 * CI, BPC], F32)

            def phase_a(b, interleave=None):
                """Reduce v[b] -> gcaty/gcatx[:, b]."""
                dscr = dram_pool.tile([CI, 2, 2 * W], F32, tag="dscr")
                drain = 0
                inter = interleave() if interleave is not None else None
                for blk in range(NBLK):
                    if inter is not None:
                        next(inter, None)
                        if blk >= NBLK // 2:
                            next(inter, None)
                    i0 = blk * IBLK
                    t = in_pool.tile([128, IBLK, 2, W], F32, tag="in")
                    nc.sync.dma_start(
                        t[:],
                        v[b, i0 : i0 + IBLK, :, :].rearrange(
                            "i (p hh) w -> p i hh w", p=HP
                        ),
                    )
                    pj = []
                    for ii in range(IBLK):
                        p = psum_p.tile([2, 2, W], F32, tag="P")
                        pj.append(p)
                        nc.tensor.matmul(
                            p[:], lhsT=sb_y2e[:], rhs=t[:, ii, :, :],
                            start=True, stop=True,
                        )
                    s_blk = scratch.tile([2, IBLK, 2 * W], F32, tag="sblk")
                    for ii in range(IBLK):
                        dst = s_blk[:, ii, :]
                        src = pj[ii][:].rearrange("c hh w -> c (hh w)")
                        if drain % 2 == 0:
                            nc.scalar.copy(dst, src)
                        else:
                            nc.vector.tensor_copy(dst, src)
                        drain += 1
                    nc.scalar.dma_start(
                        dscr[i0 : i0 + IBLK, :, :].rearrange("i p f -> p i f"),
                        s_blk[:],
                    )
                # re-partition on readback: dscr[i, p, f] -> s2[2i+p, f]
                s2 = scratch.tile([2 * CI, 2 * W], F32, tag="s2")
                nc.scalar.dma_start(s2[:], dscr[:].rearrange("i p f -> (i p) f"))
                sc2 = scratch.tile([2 * CI, 2 * W], F32, tag="sc2")
                nc.vector.tensor_tensor(out=sc2[:], in0=s2[:], in1=sb_wty[:], op=MULT)
                nc.vector.tensor_reduce(
                    out=gcaty[:, b : b + 1], in_=sc2[:],
                    axis=mybir.AxisListType.X, op=ADD,
                )
                sc3 = scratch.tile([2 * CI, 2 * W], F32, tag="sc2")
                nc.vector.tensor_tensor(out=sc3[:], in0=s2[:], in1=sb_wtx[:], op=MULT)
                nc.vector.tensor_reduce(
                    out=gcatx[:, b : b + 1], in_=sc3[:],
                    axis=mybir.AxisListType.X, op=ADD,
                )

            def tiny(b):
                """gcaty/x[:, b] -> per-partition scale/bias SBUF tiles."""
                inner_ps = psum_t.tile([1, R], F32, tag="tiny")
                nc.tensor.matmul(
                    inner_ps[:], lhsT=gcaty[:, b : b + 1], rhs=sb_psi2y[:],
                    start=True, stop=False,
                )
                nc.tensor.matmul(
                    inner_ps[:], lhsT=gcatx[:, b : b + 1], rhs=sb_psi2x[:],
                    start=False, stop=True,
                )
                sb_inner = scratch.tile([1, R], F32, tag="ti1")
                nc.vector.tensor_copy(sb_inner[:], inner_ps[:])

                innert_ps = psum_t.tile([R, 1], F32, tag="tiny")
                nc.tensor.transpose(innert_ps[:], sb_inner[:], sb_id1[:])
                sb_innert = scratch.tile([R, 1], F32, tag="ti2")
                nc.vector.tensor_copy(sb_innert[:], innert_ps[:])

                ab_ps = psum_t.tile([1, 2 * CO], F32, tag="tiny")
                nc.tensor.matmul(
                    ab_ps[:], lhsT=sb_innert[:], rhs=sb_phicat[:],
                    start=True, stop=True,
                )
                sb_ab = scratch.tile([1, 2 * CO], F32, tag="ti3")
                nc.vector.tensor_copy(sb_ab[:], ab_ps[:])

                outs = []
                for k in range(3):  # bias_even (A*y_even), bias_odd (A*y_odd), scale (B)
                    ps = psum_bc.tile([128, 2 * CO], F32, tag="bc")
                    nc.tensor.matmul(
                        ps[:],
                        lhsT=sb_ybc[0:1, 128 * k : 128 * (k + 1)],
                        rhs=sb_ab[:],
                        start=True,
                        stop=True,
                    )
                    sb = bc_pool.tile([128, 2 * CO], F32, tag="bcs")
                    nc.vector.tensor_copy(sb[:], ps[:])
                    outs.append(sb)
                return outs  # [bias_even, bias_odd, scale]

            def _phase_b_gen(b, bias_e, bias_o, scale):
                eng = 0
                for oc in range(NOBLK):
                    yield
                    ot = out_pool.tile([128, OBLK, 2, W], F32, tag="out")
                    for ol in range(OBLK):
                        o = oc * OBLK + ol
                        sc_ap = scale[:, 2 * o + 1 : 2 * o + 2]
                        for hh in range(2):
                            bias_ap = (bias_e if hh == 0 else bias_o)[:, 2 * o : 2 * o + 1]
                            dst = ot[:, ol, hh, :]
                            which = _GEN_ENGINES[eng % len(_GEN_ENGINES)]
                            eng += 1
                            if which == "dve":
                                nc.vector.tensor_scalar(
                                    out=dst, in0=sb_xrep[:], scalar1=sc_ap,
                                    scalar2=bias_ap, op0=MULT, op1=ADD,
                                )
                            elif which == "pool":
                                nc.gpsimd.tensor_scalar(
                                    out=dst, in0=sb_xrep[:], scalar1=sc_ap,
                                    scalar2=bias_ap, op0=MULT, op1=ADD,
                                )
                            else:
                                nc.scalar.activation(
                                    dst, sb_xrep[:],
                                    mybir.ActivationFunctionType.Identity,
                                    bias=bias_ap, scale=sc_ap,
                                )
                    nc.scalar.dma_start(
                        u[b, oc * OBLK : (oc + 1) * OBLK, :, :].rearrange(
                            "o (p hh) w -> p o hh w", p=128
                        ),
                        ot[:],
                    )

            phase_a(0)
            sb0 = tiny(0)
            b0_gen = _phase_b_gen(0, *sb0)
            phase_a(1, interleave=lambda: b0_gen)
            for _ in b0_gen:
                pass
            sb1 = tiny(1)
            for _ in _phase_b_gen(1, *sb1):
                pass

    nc.compile()
    return nc


def make_in_maps(v, psi, phi):
    y = np.linspace(-1.0, 1.0, H, dtype=np.float32)
    x = np.linspace(-1.0, 1.0, W, dtype=np.float32)
    dx = np.float32(2.0 / (W - 1))
    dy = np.float32(2.0 / (H - 1))
    ones = np.ones(128, dtype=np.float32)

    # psi packs: inner = sum_q gy[q]*psi2y[q, r] + gx[q]*psi2x[q, r]
    # gy[2i] + gy[2i+1] = Vy[i]; gx[2i+1] = Vx[i], gx[2i] = 0
    psi2y = np.empty((2 * CI, R), np.float32)
    psi2y[0::2, :] = psi[:, :, 0].T * dx
    psi2y[1::2, :] = psi[:, :, 0].T * dx
    psi2x = np.zeros((2 * CI, R), np.float32)
    psi2x[1::2, :] = psi[:, :, 1].T * dx

    phicat = np.stack([phi[:, :, 0].T, phi[:, :, 1].T], axis=2).reshape(R, 2 * CO)

    # reduction weights over s2[2i+p, (hh w)]:
    #  row 2i   = y_even-weighted sums -> Vy part, weight 1
    #  row 2i+1 = per-hh colsums -> Vy correction dy*[hh==1]; Vx weight x_w
    wty = np.zeros((2 * CI, 2 * W), np.float32)
    wty[0::2, :] = 1.0
    wty[1::2, W:] = dy
    wtx = np.zeros((2 * CI, 2 * W), np.float32)
    wtx[1::2, 0:W] = x
    wtx[1::2, W:] = x

    shards = np.ascontiguousarray(v.reshape(N_CORES, BPC, CI, H, W))
    common = {
        "psi2y": psi2y,
        "psi2x": psi2x,
        "phicat": np.ascontiguousarray(phicat),
        "wty": wty,
        "wtx": wtx,
        "y2e": np.stack([y[0::2], ones], axis=1).astype(np.float32),
        "xrep": np.broadcast_to(x, (128, W)).copy(),
        "ybc": np.concatenate([y[0::2], y[1::2], ones])[None, :].astype(np.float32),
        "ident1": np.ones((1, 1), dtype=np.float32),
    }
    return [{"v": shards[i], **common} for i in range(N_CORES)]


_NC_CACHE = None


def kernel(v, psi, phi):
    global _NC_CACHE
    if _NC_CACHE is None:
        _NC_CACHE = build_nc()
    nc = _NC_CACHE
    in_maps = make_in_maps(
        np.ascontiguousarray(v, dtype=np.float32),
        np.asarray(psi, dtype=np.float32),
        np.asarray(phi, dtype=np.float32),
    )
    res = run_bass_kernel_spmd(nc, in_maps, core_ids=list(range(N_CORES)))
    return np.concatenate([r["u"] for r in res.results], axis=0)


if __name__ == "__main__":
    build_nc()
    print("build ok")



# revision 3
# speedup vs baseline: 2.0498x; 2.0498x over previous
"""Trainium2 Bass kernel for the low-rank linear operator.

Math: the reference collapses algebraically. With y = linspace(-1,1,H),
x = linspace(-1,1,W), dx = 2/(W-1):

  Vy[b,i] = sum_{h,w} v[b,i,h,w] * y_h
  Vx[b,i] = sum_{h,w} v[b,i,h,w] * x_w
  inner[b,r] = dx * sum_i (Vy[b,i]*psi[r,i,0] + Vx[b,i]*psi[r,i,1])
  A[b,o] = sum_r inner[b,r]*phi[o,r,0];  Bc[b,o] = sum_r inner[b,r]*phi[o,r,1]
  u[b,o,h,w] = A[b,o]*y_h + Bc[b,o]*x_w

Sharding: data-parallel over batch, 2 batches per core, 8 cores, no
collectives.

The whole problem is HBM-bandwidth bound (read v, write u). fp16 is used
for the streamed tensors (tolerance is 2e-2; fp16 end-to-end measures
~4.5e-4), halving DMA traffic vs f32. Host pre-transposes v to
[b, q, p=h%128, i, w] so each input DMA descriptor is 16KB contiguous,
and u is produced in the mirrored layout (8KB descriptors) and
re-transposed on the host.

Reduction: h is split in two halves q (partition p = h - 128q). For each
channel ch a matmul with a sliding-window lhsT (zeros except columns
2ch -> y-half values, 2ch+1 -> ones) accumulates y-weighted row sums and
column sums for ALL 64 channels into a single [128, 256] f32 psum tile
(row 2i = sum_h y_h v[i], row 2i+1 = sum_h v[i]). One full-width
mult+reduce against [1; x] weights then yields the per-channel
(Vy, Vx) vector g[128] directly -- no DRAM bounce, no 2-partition
drains. Tiny f32 matmuls produce inner -> (A,B) -> per-partition
scale/bias tiles; DVE/ACT/Pool tensor_scalar ops generate u tiles as
x_w*B + y_h*A.
"""

import sys

try:
    import concourse.bass as bass  # noqa: F401
except ImportError:
    for _p in ("/opt/trn_rl_repo", "/root/.axon_site/_ro/trn_rl_repo"):
        if _p not in sys.path:
            sys.path.insert(0, _p)

import numpy as np

import concourse.bacc as bacc
import concourse.bass as bass
import concourse.mybir as mybir
import concourse.tile as tile
from concourse.bass_utils import run_bass_kernel_spmd

F32 = mybir.dt.float32
F16 = mybir.dt.float16
MULT = mybir.AluOpType.mult
ADD = mybir.AluOpType.add

B, CI, CO, R, H, W = 16, 64, 64, 64, 256, 256
N_CORES = 8
BPC = B // N_CORES  # batches per core
NQ = 2              # h-halves (partition p = h - 128*q)

IBLK = 32           # input channels per DMA (2MB fp16 transfers)
NIB = CI // IBLK
OBLK = 16           # output channels per DMA (1MB fp16 transfers)
NOB = CO // OBLK

# generation-engine rotation: mostly DVE (fp16 4x packed), some ACT/Pool
_GEN_ENGINES = ("dve", "dve", "act", "dve", "dve", "dve", "pool", "dve")


def build_nc():
    nc = bacc.Bacc("TRN2", target_bir_lowering=False, debug=False)

    # v4/u4 index j = 2*b + q
    v4 = nc.dram_tensor("v4", [BPC * NQ, 128, CI, W], F16, kind="ExternalInput")
    ylhs = nc.dram_tensor("ylhs", [128, 2 * 384], F16, kind="ExternalInput")
    wcat = nc.dram_tensor("wcat", [128, W], F32, kind="ExternalInput")
    psicat2 = nc.dram_tensor("psicat2", [2 * CI, R], F32, kind="ExternalInput")
    phicat = nc.dram_tensor("phicat", [R, 2 * CO], F32, kind="ExternalInput")
    ybc = nc.dram_tensor("ybc", [1, 384], F32, kind="ExternalInput")
    xrep = nc.dram_tensor("xrep", [128, W], F16, kind="ExternalInput")
    ident1 = nc.dram_tensor("ident1", [1, 1], F32, kind="ExternalInput")
    u4 = nc.dram_tensor("u4", [BPC * NQ, 128, CO, W], F16, kind="ExternalOutput")

    with tile.TileContext(nc) as tc:
        with (
            tc.tile_pool(name="consts", bufs=1) as consts,
            tc.tile_pool(name="inp", bufs=3) as in_pool,
            tc.tile_pool(name="outp", bufs=4) as out_pool,
            tc.tile_pool(name="scr", bufs=3) as scratch,
            tc.tile_pool(name="bc", bufs=6) as bc_pool,
            tc.tile_pool(name="psumA", bufs=2, space="PSUM") as psum_a,
            tc.tile_pool(name="psumT", bufs=1, space="PSUM") as psum_t,
            tc.tile_pool(name="psumBC", bufs=2, space="PSUM") as psum_bc,
        ):
            sb_ylhs = consts.tile([128, 2 * 384], F16)
            nc.sync.dma_start(sb_ylhs[:], ylhs[:])
            sb_wcat = consts.tile([128, W], F32)
            nc.sync.dma_start(sb_wcat[:], wcat[:])
            sb_psicat2 = consts.tile([2 * CI, R], F32)
            nc.sync.dma_start(sb_psicat2[:], psicat2[:])
            sb_phicat = consts.tile([R, 2 * CO], F32)
            nc.sync.dma_start(sb_phicat[:], phicat[:])
            sb_ybc = consts.tile([1, 384], F32)
            nc.sync.dma_start(sb_ybc[:], ybc[:])
            sb_xrep = consts.tile([128, W], F16)
            nc.sync.dma_start(sb_xrep[:], xrep[:])
            sb_id1 = consts.tile([1, 1], F32)
            nc.sync.dma_start(sb_id1[:], ident1[:])

            # per-batch (Vy|Vx interleaved) reduction vector, one column per batch
            g_sb = consts.tile([2 * CI, BPC], F32)

            def stage_a(b):
                """Reduce v[b] -> g_sb[:, b] (rows 2i = Vy[i], 2i+1 = Vx[i])."""
                ps = psum_a.tile([128, W], F32, tag="A")
                for q in range(NQ):
                    for blk in range(NIB):
                        t = in_pool.tile([128, IBLK, W], F16, tag="in")
                        nc.sync.dma_start(
                            t[:],
                            v4[2 * b + q, :, blk * IBLK : (blk + 1) * IBLK, :],
                        )
                        for ii in range(IBLK):
                            ch = blk * IBLK + ii
                            lo = 384 * q + 128 - 2 * ch
                            nc.tensor.matmul(
                                ps[:],
                                lhsT=sb_ylhs[:, lo : lo + 128],
                                rhs=t[:, ii, :],
                                start=(q == 0 and ch == 0),
                                stop=(q == 1 and ch == CI - 1),
                            )
                s2 = scratch.tile([128, W], F32, tag="s2")
                nc.scalar.copy(s2[:], ps[:])
                sc = scratch.tile([128, W], F32, tag="sc")
                nc.vector.tensor_tensor(out=sc[:], in0=s2[:], in1=sb_wcat[:], op=MULT)
                nc.vector.tensor_reduce(
                    out=g_sb[:, b : b + 1], in_=sc[:],
                    axis=mybir.AxisListType.X, op=ADD,
                )

            def tiny(b):
                """g_sb[:, b] -> per-partition scale/bias SBUF tiles (fp16)."""
                inner_ps = psum_t.tile([1, R], F32, tag="tiny")
                nc.tensor.matmul(
                    inner_ps[:], lhsT=g_sb[:, b : b + 1], rhs=sb_psicat2[:],
                    start=True, stop=True,
                )
                sb_inner = scratch.tile([1, R], F32, tag="ti1")
                nc.vector.tensor_copy(sb_inner[:], inner_ps[:])

                innert_ps = psum_t.tile([R, 1], F32, tag="tiny")
                nc.tensor.transpose(innert_ps[:], sb_inner[:], sb_id1[:])
                sb_innert = scratch.tile([R, 1], F32, tag="ti2")
                nc.vector.tensor_copy(sb_innert[:], innert_ps[:])

                ab_ps = psum_t.tile([1, 2 * CO], F32, tag="tiny")
                nc.tensor.matmul(
                    ab_ps[:], lhsT=sb_innert[:], rhs=sb_phicat[:],
                    start=True, stop=True,
                )
                sb_ab = scratch.tile([1, 2 * CO], F32, tag="ti3")
                nc.vector.tensor_copy(sb_ab[:], ab_ps[:])

                outs = []
                for k in range(3):  # bias_q0 (A*y_q0), bias_q1 (A*y_q1), scale (B)
                    ps = psum_bc.tile([128, 2 * CO], F32, tag="bc")
                    nc.tensor.matmul(
                        ps[:],
                        lhsT=sb_ybc[0:1, 128 * k : 128 * (k + 1)],
                        rhs=sb_ab[:],
                        start=True,
                        stop=True,
                    )
                    sb = bc_pool.tile([128, 2 * CO], F32, tag="bcs")
                    nc.vector.tensor_copy(sb[:], ps[:])
                    outs.append(sb)
                return outs  # [bias_q0, bias_q1, scale]

            def stage_c(b, bias_q, scale):
                eng = 0
                for q in range(NQ):
                    for oc in range(NOB):
                        ot = out_pool.tile([128, OBLK, W], F16, tag="out")
                        for ol in range(OBLK):
                            o = oc * OBLK + ol
                            sc_ap = scale[:, 2 * o + 1 : 2 * o + 2]
                            bias_ap = bias_q[q][:, 2 * o : 2 * o + 1]
                            dst = ot[:, ol, :]
                            which = _GEN_ENGINES[eng % len(_GEN_ENGINES)]
                            eng += 1
                            if which == "dve":
                                nc.vector.tensor_scalar(
                                    out=dst, in0=sb_xrep[:], scalar1=sc_ap,
                                    scalar2=bias_ap, op0=MULT, op1=ADD,
                                )
                            elif which == "pool":
                                nc.gpsimd.tensor_scalar(
                                    out=dst, in0=sb_xrep[:], scalar1=sc_ap,
                                    scalar2=bias_ap, op0=MULT, op1=ADD,
                                )
                            else:
                                nc.scalar.activation(
                                    dst, sb_xrep[:],
                                    mybir.ActivationFunctionType.Identity,
                                    bias=bias_ap, scale=sc_ap,
                                )
                        nc.scalar.dma_start(
                            u4[2 * b + q, :, oc * OBLK : (oc + 1) * OBLK, :],
                            ot[:],
                        )

            for b in range(BPC):
                stage_a(b)
                b0, b1, sc = tiny(b)
                stage_c(b, (b0, b1), sc)

    nc.compile()
    return nc


def make_in_maps(v, psi, phi):
    f16 = np.float16
    y = np.linspace(-1.0, 1.0, H, dtype=np.float32)
    x = np.linspace(-1.0, 1.0, W, dtype=np.float32)
    dx = np.float32(2.0 / (W - 1))

    # sliding-window lhsT: for channel ch use cols [384q+128-2ch, +128)
    ylhs = np.zeros((128, 2 * 384), np.float32)
    ylhs[:, 128] = y[:128]
    ylhs[:, 129] = 1.0
    ylhs[:, 384 + 128] = y[128:]
    ylhs[:, 384 + 129] = 1.0

    wcat = np.empty((128, W), np.float32)
    wcat[0::2, :] = 1.0
    wcat[1::2, :] = x

    psicat2 = np.empty((2 * CI, R), np.float32)
    psicat2[0::2, :] = psi[:, :, 0].T * dx
    psicat2[1::2, :] = psi[:, :, 1].T * dx

    phicat = np.stack([phi[:, :, 0].T, phi[:, :, 1].T], axis=2).reshape(R, 2 * CO)

    # v[b, i, h, w] -> [b, q, p, i, w] fp16, then per-core [4, 128, CI, W]
    v16 = v.astype(f16)
    vt = v16.reshape(B, CI, NQ, 128, W).transpose(0, 2, 3, 1, 4)

    common = {
        "ylhs": ylhs.astype(f16),
        "wcat": wcat,
        "psicat2": psicat2,
        "phicat": np.ascontiguousarray(phicat),
        "ybc": np.concatenate([y[:128], y[128:], np.ones(128, np.float32)])[None, :],
        "xrep": np.broadcast_to(x.astype(f16), (128, W)).copy(),
        "ident1": np.ones((1, 1), dtype=np.float32),
    }
    return [
        {
            "v4": np.ascontiguousarray(
                vt[2 * c : 2 * c + 2].reshape(BPC * NQ, 128, CI, W)
            ),
            **common,
        }
        for c in range(N_CORES)
    ]


def gather_out(results):
    """Per-core u4 [4, 128, CO, W] fp16 -> full u [B, CO, H, W] f32."""
    arr = np.stack([r["u4"] for r in results])  # [8, 4, 128, CO, W]
    arr = arr.reshape(N_CORES, BPC, NQ, 128, CO, W).transpose(0, 1, 4, 2, 3, 5)
    return np.ascontiguousarray(
        arr.reshape(B, CO, H, W).astype(np.float32)
    )


_NC_CACHE = None


def kernel(v, psi, phi):
    global _NC_CACHE
    if _NC_CACHE is None:
        _NC_CACHE = build_nc()
    nc = _NC_CACHE
    in_maps = make_in_maps(
        np.asarray(v, dtype=np.float32),
        np.asarray(psi, dtype=np.float32),
        np.asarray(phi, dtype=np.float32),
    )
    res = run_bass_kernel_spmd(nc, in_maps, core_ids=list(range(N_CORES)))
    return gather_out(res.results)


if __name__ == "__main__":
    build_nc()
    print("build ok")
